# revision 5
# baseline (speedup 1.0000x reference)
"""AttentionSubsample as a hand-written Bass/Tile kernel on 8 trn2 cores.

Data-parallel over batch (8 samples/core). BN algebra is folded:
  * k-side BN shift cancels in softmax; its scale folds into q_eff.
  * v-side BN becomes a per-feature affine on the attention output.
  * exact global stats via one tiny AllReduce after the kv/q matmuls and
    one after the proj matmul.

Layouts (per core):
  xT   [512, 6272]  bf16   x^T, token t = b*784 + n
  xqT  [512, 1568]  bf16   subsampled tokens, t = b*196 + q
  kT   [4x128, 6272] bf16  k features f = h*32+d  (feature-major)
  v    per (b, jt) [128|16, 16*65] bf16, head block = 64 v-cols + ones col
  expB [16x128, 1372] bf16 exp(bias)^T; col-block kt, row p -> ktok 128*kt+p
  outT [8x128, 1568] bf16  attention out^T, feature f = h*64+d (=128*hp+...)
  out  [1568, 768]  f32    final, token-major
"""

import sys

sys.path.insert(0, "/opt/trn_rl_repo")

from contextlib import ExitStack

import numpy as np

import concourse.bass as bass
import concourse.mybir as mybir
import concourse.tile as tile
from concourse.bass2jax import bass_jit
from concourse.masks import make_identity

F32 = mybir.dt.float32
BF16 = mybir.dt.bfloat16
ALU = mybir.AluOpType
ACT = mybir.ActivationFunctionType

RES, RES_, STRIDE = 28, 14, 2
H, KD, D = 16, 32, 64
EPS = 1e-5
NCORES = 8
B_LOC = 8                       # samples per core
NTOK = B_LOC * 784              # 6272 kv tokens per core
NQ = B_LOC * 196                # 1568 q tokens per core
M_KV = 64 * 784                 # global token counts for BN stats
M_Q = 64 * 196
KAPPA = float(KD) ** -0.5

# kv-token tiling within a sample: 784 = 6*128 + 16
KT_CNT = [128, 128, 128, 128, 128, 128, 16]
N_KT = 7
# q-token blocks: 1568 = 4*392 ; kv eviction blocks: 784 = 2*392
QTB, KVTB = 392, 392
# proj out: 768 = 6*128 ; act features: 1024 = 8*128 ; q/k feats: 512 = 4*128
NF_P, NF_A, NF_K = 6, 8, 4


def _attn_body(nc: bass.Bass, xT, xqT, wkT, wvT, wqT, wpT, expB, gb,
               collectives=True):
    out = nc.dram_tensor("out_tokmajor", [NQ, 768], F32, kind="ExternalOutput")

    def _allreduce(in_t, out_t, rg):
        if collectives:
            nc.gpsimd.collective_compute(
                "AllReduce", ALU.add, replica_groups=rg,
                ins=[in_t[:, :].opt()], outs=[out_t[:, :].opt()])
        else:
            nc.gpsimd.dma_start(out_t[:, :], in_t[:, :])

    with tile.TileContext(nc) as tc, ExitStack() as ctx:
        const = ctx.enter_context(tc.tile_pool(name="const", bufs=1))
        dram = ctx.enter_context(tc.tile_pool(name="dram", bufs=1, space="DRAM"))
        persist = ctx.enter_context(tc.tile_pool(name="persist", bufs=1))

        # ---- constants ----------------------------------------------------
        ident = const.tile([128, 128], BF16)
        make_identity(nc, ident)
        # selector rows for the denominator-replicate matmuls, data on
        # partition 64 (must match the rhs partition base).
        sel = const.tile([128, 256], BF16)
        nc.vector.memset(sel, 0.0)
        nc.vector.memset(sel[64:65, 0:64], 1.0)      # selA: out parts 0-63
        nc.vector.memset(sel[64:65, 192:256], 1.0)   # selB: out parts 64-127
        gb_sb = const.tile([128, 40], F32)
        nc.sync.dma_start(gb_sb, gb[:, :])
        half = const.tile([128, 1], F32)
        nc.vector.memset(half, 0.5)

        # ---- persistent sbuf ---------------------------------------------
        kT_sb = [persist.tile([128, NTOK], BF16, name=f"kT{i}") for i in range(NF_K)]
        q_sb = [persist.tile([128, NQ], BF16, name=f"q{i}") for i in range(NF_K)]
        outT_sb = [persist.tile([128, NQ], BF16, name=f"oT{i}") for i in range(NF_A)]
        # stat vectors
        stats = const.tile([128, 32], F32)           # local sums -> AR1 input
        sS = const.tile([128, 26], F32)              # scratch for finalize
        Seff = const.tile([128, NF_K], F32)
        Ceff = const.tile([128, NF_K], F32)
        s_v = const.tile([128, NF_A], F32)
        c_v = const.tile([128, NF_A], F32)
        s_p = const.tile([128, NF_P], F32)
        c_p = const.tile([128, NF_P], F32)

        # dram spill for token-major v and the collective bounce buffers
        vspill = dram.tile([B_LOC, N_KT, 128, 1024], BF16)
        ar1_in = dram.tile([128, 32], F32)
        ar1_out = dram.tile([128, 32], F32)
        ar2_in = dram.tile([128, 12], F32)
        ar2_out = dram.tile([128, 12], F32)

        rg = [list(range(NCORES))]

        # ============== PHASE A: q and kv matmuls + local stats ===========
        with (
            tc.tile_pool(name="sbA", bufs=1) as sbA,
            tc.tile_pool(name="psA", bufs=1, space="PSUM") as psA,
        ):
            xqT_sb = [sbA.tile([128, NQ], BF16, name=f"xq{c}") for c in range(4)]
            wqT_sb = [sbA.tile([128, 512], BF16, name=f"wq{c}") for c in range(4)]
            xT_sb = [sbA.tile([128, NTOK], BF16, name=f"xt{c}") for c in range(4)]
            wkT_sb = [sbA.tile([128, 512], BF16, name=f"wk{c}") for c in range(4)]
            wvT_sb = [sbA.tile([128, 1024], BF16, name=f"wv{c}") for c in range(4)]
            for c in range(4):
                nc.sync.dma_start(xqT_sb[c], xqT[128 * c:128 * (c + 1), :])
                nc.sync.dma_start(wqT_sb[c], wqT[128 * c:128 * (c + 1), :])
                nc.sync.dma_start(xT_sb[c], xT[128 * c:128 * (c + 1), :])
                nc.sync.dma_start(wkT_sb[c], wkT[128 * c:128 * (c + 1), :])
                nc.sync.dma_start(wvT_sb[c], wvT[128 * c:128 * (c + 1), :])

            qstat = sbA.tile([128, NF_K, 4, 6], F32)
            kstat = sbA.tile([128, NF_K, 16, 6], F32)
            vstat = sbA.tile([128, NF_A, 16, 6], F32)

            # ---- q = xq @ Wq^T (feature-major) ----
            for ft in range(NF_K):
                for tb in range(4):
                    qps = psA.tile([128, QTB], F32, tag="mmps", bufs=3)
                    for c in range(4):
                        nc.tensor.matmul(
                            qps, lhsT=wqT_sb[c][:, 128 * ft:128 * (ft + 1)],
                            rhs=xqT_sb[c][:, QTB * tb:QTB * (tb + 1)],
                            start=(c == 0), stop=(c == 3))
                    nc.scalar.copy(q_sb[ft][:, QTB * tb:QTB * (tb + 1)], qps)
                    nc.vector.bn_stats(qstat[:, ft, tb, :], qps)

            # ---- kv = x @ Wkv^T (k feature-major; v transposed + spilled) --
            for b in range(B_LOC):
                vTb = [sbA.tile([128, 784], BF16, tag=f"vTb{f}", bufs=2,
                                name=f"vTb{f}") for f in range(NF_A)]
                for ft in range(12):
                    for tb in range(2):
                        ps = psA.tile([128, KVTB], F32, tag="mmps", bufs=3)
                        w = wkT_sb if ft < NF_K else wvT_sb
                        fo = ft if ft < NF_K else ft - NF_K
                        for c in range(4):
                            nc.tensor.matmul(
                                ps, lhsT=w[c][:, 128 * fo:128 * (fo + 1)],
                                rhs=xT_sb[c][:, 784 * b + KVTB * tb:
                                             784 * b + KVTB * (tb + 1)],
                                start=(c == 0), stop=(c == 3))
                        if ft < NF_K:
                            nc.scalar.copy(
                                kT_sb[ft][:, 784 * b + KVTB * tb:
                                          784 * b + KVTB * (tb + 1)], ps)
                            nc.vector.bn_stats(kstat[:, ft, 2 * b + tb, :], ps)
                        else:
                            nc.scalar.copy(
                                vTb[fo][:, KVTB * tb:KVTB * (tb + 1)], ps)
                            nc.vector.bn_stats(vstat[:, fo, 2 * b + tb, :], ps)
                # transpose v for this sample: [128f, tok] -> [tok, 1024]
                for jt in range(N_KT):
                    cnt = KT_CNT[jt]
                    vps = psA.tile([128, 1024], BF16, tag="vtr", bufs=2)
                    for fv in range(NF_A):
                        nc.tensor.transpose(
                            vps[0:cnt, 128 * fv:128 * (fv + 1)],
                            vTb[fv][:, 128 * jt:128 * jt + cnt], ident)
                    vtmp = sbA.tile([128, 1024], BF16, tag="vtmp", bufs=3)
                    nc.vector.tensor_copy(vtmp[0:cnt, :], vps[0:cnt, :])
                    nc.sync.dma_start(vspill[b, jt, 0:cnt, :], vtmp[0:cnt, :])

            # ---- local stats -> sums -> AR1 input ----
            # layout of `stats`: qS 0:4 | qS2 4:8 | kS 8:12 | kS2 12:16
            #                    vS 16:24 | vS2 24:32
            mv = sbA.tile([128, 32], F32)  # (mean,var) pairs per ftile
            for ft in range(NF_K):
                nc.vector.bn_aggr(mv[:, 2 * ft:2 * ft + 2], qstat[:, ft, :, :])
            for ft in range(NF_K):
                nc.vector.bn_aggr(mv[:, 8 + 2 * ft:8 + 2 * ft + 2],
                                  kstat[:, ft, :, :])
            for ft in range(NF_A):
                nc.vector.bn_aggr(mv[:, 16 + 2 * ft:16 + 2 * ft + 2],
                                  vstat[:, ft, :, :])
            # means at even cols, vars at odd cols of mv
            m_all = mv[:, 0:32:2]
            v_all = mv[:, 1:32:2]
            # sums: S = m*Mloc ; S2 = (var + m^2)*Mloc
            nloc = sbA.tile([128, 16], F32)
            nc.vector.memset(nloc[:, 0:4], float(NQ))
            nc.vector.memset(nloc[:, 4:16], float(NTOK))
            s_cols = stats[:, 0:4], stats[:, 8:12], stats[:, 16:24]
            s2_cols = stats[:, 4:8], stats[:, 12:16], stats[:, 24:32]
            m_grp = m_all[:, 0:4], m_all[:, 4:8], m_all[:, 8:16]
            v_grp = v_all[:, 0:4], v_all[:, 4:8], v_all[:, 8:16]
            n_grp = nloc[:, 0:4], nloc[:, 4:8], nloc[:, 8:16]
            for s_c, s2_c, m_c, v_c, n_c in zip(
                    s_cols, s2_cols, m_grp, v_grp, n_grp):
                nc.vector.tensor_tensor(s_c, m_c, n_c, ALU.mult)
                # tmp = var + m^2
                nc.vector.tensor_tensor(s2_c, m_c, m_c, ALU.mult)
                nc.vector.tensor_tensor(s2_c, s2_c, v_c, ALU.add)
                nc.vector.tensor_tensor(s2_c, s2_c, n_c, ALU.mult)

            nc.gpsimd.dma_start(ar1_in[:, :], stats)
            _allreduce(ar1_in, ar1_out, rg)
            arr = sbA.tile([128, 32], F32)
            nc.gpsimd.dma_start(arr, ar1_out[:, :])

            # ---- finalize: istd etc.  sS layout: m 0:16 | istd 16:32->reuse
            inv_m = sbA.tile([128, 16], F32)
            nc.vector.memset(inv_m[:, 0:4], 1.0 / M_Q)
            nc.vector.memset(inv_m[:, 4:16], 1.0 / M_KV)
            mg = sS[:, 0:16]
            ist = sS[:, 16:26]  # unused tail; real slices below
            g_s = arr[:, 0:4], arr[:, 8:12], arr[:, 16:24]
            g_s2 = arr[:, 4:8], arr[:, 12:16], arr[:, 24:32]
            g_m = mg[:, 0:4], mg[:, 4:8], mg[:, 8:16]
            g_im = inv_m[:, 0:4], inv_m[:, 4:8], inv_m[:, 8:16]
            var_t = sbA.tile([128, 16], F32)
            g_var = var_t[:, 0:4], var_t[:, 4:8], var_t[:, 8:16]
            for s_c, s2_c, m_c, im_c, va_c in zip(
                    g_s, g_s2, g_m, g_im, g_var):
                nc.vector.tensor_tensor(m_c, s_c, im_c, ALU.mult)
                nc.vector.tensor_tensor(va_c, s2_c, im_c, ALU.mult)
                t2 = sbA.tile([128, 8], F32, tag="fint", bufs=4)
                nc.vector.tensor_tensor(t2[:, 0:m_c.shape[1]], m_c, m_c,
                                        ALU.mult)
                nc.vector.tensor_tensor(va_c, va_c, t2[:, 0:m_c.shape[1]],
                                        ALU.subtract)
            istd = sbA.tile([128, 16], F32)
            nc.vector.tensor_scalar(var_t, var_t, float(EPS), None, ALU.add)
            nc.scalar.activation(istd, var_t, ACT.Sqrt)
            nc.vector.reciprocal(istd, istd)
            # q_eff = (q*s_q + c_q) * (s_k * kappa)
            #   s_q = g_q*istd_q ; c_q = b_q - s_q*m_q ; skk = g_k*istd_k*kappa
            sq = sbA.tile([128, 4], F32)
            cq = sbA.tile([128, 4], F32)
            skk = sbA.tile([128, 4], F32)
            nc.vector.tensor_tensor(sq, gb_sb[:, 0:4], istd[:, 0:4], ALU.mult)
            nc.vector.tensor_tensor(cq, sq, mg[:, 0:4], ALU.mult)
            nc.vector.tensor_tensor(cq, gb_sb[:, 4:8], cq, ALU.subtract)
            nc.vector.tensor_tensor(skk, gb_sb[:, 8:12], istd[:, 4:8], ALU.mult)
            nc.vector.tensor_scalar(skk, skk, KAPPA, None, ALU.mult)
            nc.vector.tensor_tensor(Seff, sq, skk, ALU.mult)
            nc.vector.tensor_tensor(Ceff, cq, skk, ALU.mult)
            # v affine: s_v = g_v*istd_v ; c_v = b_v - s_v*m_v
            nc.vector.tensor_tensor(s_v, gb_sb[:, 12:20], istd[:, 8:16],
                                    ALU.mult)
            nc.vector.tensor_tensor(c_v, s_v, mg[:, 8:16], ALU.mult)
            nc.vector.tensor_tensor(c_v, gb_sb[:, 20:28], c_v, ALU.subtract)
            # apply q_eff in place
            for ft in range(NF_K):
                nc.vector.tensor_scalar(
                    q_sb[ft], q_sb[ft], Seff[:, ft:ft + 1], Ceff[:, ft:ft + 1],
                    ALU.mult, ALU.add)

        # ============== PHASE B: attention ================================
        with (
            tc.tile_pool(name="sbB", bufs=1) as sbB,
            tc.tile_pool(name="psB", bufs=1, space="PSUM") as psB,
        ):
            expB_sb = [sbB.tile([128, 1372], BF16, name=f"eB{h}")
                       for h in range(H)]
            for h in range(H):
                nc.sync.dma_start(expB_sb[h], expB[128 * h:128 * (h + 1), :])

            vring = [[sbB.tile([128, 16 * 65], BF16, tag=f"vr{jt}", bufs=2,
                               name=f"vr{jt}_{b}") for jt in range(N_KT)]
                     for b in range(2)]

            def load_v(b):
                sl = b % 2
                for jt in range(N_KT):
                    cnt = KT_CNT[jt]
                    t = vring[sl][jt]
                    nc.sync.dma_start(
                        t.rearrange("p (a c) -> p a c", c=65)[0:cnt, :, 0:64],
                        vspill[b, jt, 0:cnt, :].rearrange(
                            "p (a c) -> p a c", c=64))
                    nc.vector.memset(
                        t.rearrange("p (a c) -> p a c", c=65)[:, :, 64:65], 1.0)

            load_v(0)
            for b in range(B_LOC):
                if b + 1 < B_LOC:
                    load_v(b + 1)
                S_prev = None
                for h in range(H):
                    lgA = psB.tile([128, 1024], F32, tag="lgA", bufs=1)
                    lgB_ = psB.tile([128, 768], F32, tag="lgB", bufs=1)
                    for kt in range(N_KT):
                        cnt = KT_CNT[kt]
                        dst = (lgA[0:cnt, 256 * kt:256 * kt + 196]
                               if kt < 4 else
                               lgB_[0:cnt, 256 * (kt - 4):256 * (kt - 4) + 196])
                        hb = 32 * (h % 4)
                        nc.tensor.matmul(
                            dst,
                            lhsT=kT_sb[h // 4][hb:hb + 32,
                                               784 * b + 128 * kt:
                                               784 * b + 128 * kt + cnt],
                            rhs=q_sb[h // 4][hb:hb + 32,
                                             196 * b:196 * (b + 1)],
                            start=True, stop=True, tile_position=(hb, 0))
                    expL = sbB.tile([128, 1372], BF16, tag="expL", bufs=3)
                    nc.scalar.activation(
                        expL[:, 0:784].rearrange("p (a c) -> p a c", c=196),
                        lgA.rearrange("p (a c) -> p a c", c=256)[:, :, 0:196],
                        ACT.Exp)
                    nc.scalar.activation(
                        expL[:, 784:1372].rearrange("p (a c) -> p a c", c=196),
                        lgB_.rearrange("p (a c) -> p a c", c=256)[:, :, 0:196],
                        ACT.Exp)
                    expU = sbB.tile([128, 1372], BF16, tag="expU", bufs=3)
                    nc.vector.tensor_tensor(expU, expL, expB_sb[h], ALU.mult)
                    avps = psB.tile([65, 196], F32, tag="av", bufs=2)
                    for kt in range(N_KT):
                        cnt = KT_CNT[kt]
                        nc.tensor.matmul(
                            avps, lhsT=vring[b % 2][kt][0:cnt, 65 * h:65 * h + 65],
                            rhs=expU[0:cnt, 196 * kt:196 * (kt + 1)],
                            start=(kt == 0), stop=(kt == N_KT - 1))
                    S = sbB.tile([65, 196], BF16, tag="S", bufs=6)
                    nc.scalar.copy(S, avps)
                    hp = h // 2
                    nc.sync.dma_start(
                        outT_sb[hp][64 * (h % 2):64 * (h % 2) + 64,
                                    196 * b:196 * (b + 1)], S[0:64, :])
                    if h % 2 == 0:
                        S_prev = S
                        continue
                    # denominator replicate + normalize + v-affine + hswish
                    rep = psB.tile([128, 196], F32, tag="rep", bufs=2)
                    nc.tensor.matmul(rep, lhsT=sel[64:65, 0:128],
                                     rhs=S_prev[64:65, :],
                                     start=True, stop=False)
                    nc.tensor.matmul(rep, lhsT=sel[64:65, 128:256],
                                     rhs=S[64:65, :],
                                     start=False, stop=True)
                    recip = sbB.tile([128, 196], F32, tag="recip", bufs=2)
                    nc.vector.reciprocal_approx_fast(recip, rep)
                    sl = outT_sb[hp][:, 196 * b:196 * (b + 1)]
                    nc.vector.scalar_tensor_tensor(
                        sl, sl, s_v[:, hp:hp + 1], recip, ALU.mult, ALU.mult)
                    nc.vector.tensor_scalar(sl, sl, c_v[:, hp:hp + 1], None,
                                            ALU.add)
                    r1 = sbB.tile([128, 196], BF16, tag="hsw", bufs=2)
                    nc.scalar.activation(r1, sl, ACT.Relu, bias=half[:, 0:1],
                                         scale=1.0 / 6.0)
                    nc.vector.tensor_scalar(r1, r1, 1.0, None, ALU.min)
                    nc.vector.tensor_tensor(sl, sl, r1, ALU.mult)

        # ============== PHASE C: proj + BN + transpose + out ==============
        with (
            tc.tile_pool(name="sbC", bufs=1) as sbC,
            tc.tile_pool(name="psC", bufs=1, space="PSUM") as psC,
        ):
            wpT_sb = [sbC.tile([128, 768], BF16, name=f"wp{c}")
                      for c in range(NF_A)]
            for c in range(NF_A):
                nc.sync.dma_start(wpT_sb[c], wpT[128 * c:128 * (c + 1), :])
            projT = [sbC.tile([128, NQ], BF16, name=f"pj{i}")
                     for i in range(NF_P)]
            pstat = sbC.tile([128, NF_P, 4, 6], F32)
            for nf in range(NF_P):
                for tb in range(4):
                    pps = psC.tile([128, QTB], F32, tag="pps", bufs=3)
                    for hp in range(NF_A):
                        nc.tensor.matmul(
                            pps, lhsT=wpT_sb[hp][:, 128 * nf:128 * (nf + 1)],
                            rhs=outT_sb[hp][:, QTB * tb:QTB * (tb + 1)],
                            start=(hp == 0), stop=(hp == NF_A - 1))
                    nc.scalar.copy(projT[nf][:, QTB * tb:QTB * (tb + 1)], pps)
                    nc.vector.bn_stats(pstat[:, nf, tb, :], pps)
            # stats -> sums -> AR2 -> affine consts
            pmv = sbC.tile([128, NF_P, 2], F32)
            for nf in range(NF_P):
                nc.vector.bn_aggr(pmv[:, nf, :], pstat[:, nf, :, :])
            psum_t = sbC.tile([128, 12], F32)
            nc.vector.tensor_scalar(
                psum_t[:, 0:6], pmv[:, :, 0], float(NQ), None, ALU.mult)
            nc.vector.tensor_tensor(
                psum_t[:, 6:12], pmv[:, :, 0], pmv[:, :, 0], ALU.mult)
            nc.vector.tensor_tensor(
                psum_t[:, 6:12], psum_t[:, 6:12], pmv[:, :, 1], ALU.add)
            nc.vector.tensor_scalar(
                psum_t[:, 6:12], psum_t[:, 6:12], float(NQ), None, ALU.mult)
            nc.gpsimd.dma_start(ar2_in[:, :], psum_t)
            _allreduce(ar2_in, ar2_out, rg)
            arr2 = sbC.tile([128, 12], F32)
            nc.gpsimd.dma_start(arr2, ar2_out[:, :])
            pm = sbC.tile([128, 6], F32)
            pvar = sbC.tile([128, 6], F32)
            nc.vector.tensor_scalar(pm, arr2[:, 0:6], 1.0 / M_Q, None, ALU.mult)
            nc.vector.tensor_scalar(pvar, arr2[:, 6:12], 1.0 / M_Q, None,
                                    ALU.mult)
            pt = sbC.tile([128, 6], F32)
            nc.vector.tensor_tensor(pt, pm, pm, ALU.mult)
            nc.vector.tensor_tensor(pvar, pvar, pt, ALU.subtract)
            nc.vector.tensor_scalar(pvar, pvar, float(EPS), None, ALU.add)
            pistd = sbC.tile([128, 6], F32)
            nc.scalar.activation(pistd, pvar, ACT.Sqrt)
            nc.vector.reciprocal(pistd, pistd)
            nc.vector.tensor_tensor(s_p, gb_sb[:, 28:34], pistd, ALU.mult)
            nc.vector.tensor_tensor(c_p, s_p, pm, ALU.mult)
            nc.vector.tensor_tensor(c_p, gb_sb[:, 34:40], c_p, ALU.subtract)
            for nf in range(NF_P):
                nc.vector.tensor_scalar(
                    projT[nf], projT[nf], s_p[:, nf:nf + 1], c_p[:, nf:nf + 1],
                    ALU.mult, ALU.add)
            # transpose to token-major and write out
            ttcnt = [128] * 12 + [32]
            for tt in range(13):
                cnt = ttcnt[tt]
                tps = psC.tile([128, 768], BF16, tag="tps", bufs=2)
                for nf in range(NF_P):
                    nc.tensor.transpose(
                        tps[0:cnt, 128 * nf:128 * (nf + 1)],
                        projT[nf][:, 128 * tt:128 * tt + cnt], ident)
                osb = sbC.tile([128, 768], F32, tag="osb", bufs=2)
                nc.scalar.copy(osb[0:cnt, :], tps[0:cnt, :])
                nc.sync.dma_start(out[128 * tt:128 * tt + cnt, :],
                                  osb[0:cnt, :])

    return (out,)


_bass_fn = bass_jit(_attn_body, num_devices=NCORES)


# ======================= host side ====================================

import jax
import jax.numpy as jnp
import ml_dtypes
from jax.sharding import Mesh, PartitionSpec as P, NamedSharding

_IDX_K = np.array([h * 96 + i for h in range(H) for i in range(KD)])
_IDX_V = np.array([h * 96 + KD + j for h in range(H) for j in range(D)])


def _pack_cols(vec, nf):
    return np.ascontiguousarray(vec.reshape(nf, 128).T.astype(np.float32))


def _stage(kw):
    x = np.asarray(kw["x"], np.float32)           # [64, 784, 512]
    B = x.shape[0]
    xq = x.reshape(B, RES, RES, 512)[:, ::STRIDE, ::STRIDE]
    xq = xq.reshape(B, 196, 512)
    bf = ml_dtypes.bfloat16
    # per-core transposes, concatenated on axis 0
    xT = np.concatenate(
        [x[8 * c:8 * (c + 1)].reshape(-1, 512).T for c in range(NCORES)], 0)
    xqT = np.concatenate(
        [xq[8 * c:8 * (c + 1)].reshape(-1, 512).T for c in range(NCORES)], 0)
    W_kv = np.asarray(kw["W_kv"], np.float32)
    wkT = W_kv[_IDX_K].T                          # [512, 512]
    wvT = W_kv[_IDX_V].T                          # [512, 1024]
    wqT = np.asarray(kw["W_q"], np.float32).T     # [512, 512]
    wpT = np.asarray(kw["W_proj"], np.float32).T  # [1024, 768]
    eb = np.exp(np.asarray(kw["attn_biases"], np.float32)
                [:, np.asarray(kw["bias_idxs"])])  # [16, 196, 784]
    ebT = np.ones((H, 896, 196), np.float32)
    ebT[:, 0:784, :] = eb.transpose(0, 2, 1)
    expB = ebT.reshape(H, N_KT, 128, 196).transpose(0, 2, 1, 3).reshape(
        H * 128, N_KT * 196)
    gb = np.zeros((128, 40), np.float32)
    gb[:, 0:4] = _pack_cols(np.asarray(kw["g_q"]), 4)
    gb[:, 4:8] = _pack_cols(np.asarray(kw["b_q"]), 4)
    gb[:, 8:12] = _pack_cols(np.asarray(kw["g_kv"])[_IDX_K], 4)
    gb[:, 12:20] = _pack_cols(np.asarray(kw["g_kv"])[_IDX_V], 8)
    gb[:, 20:28] = _pack_cols(np.asarray(kw["b_kv"])[_IDX_V], 8)
    gb[:, 28:34] = _pack_cols(np.asarray(kw["g_proj"]), 6)
    gb[:, 34:40] = _pack_cols(np.asarray(kw["b_proj"]), 6)
    return (xT.astype(bf), xqT.astype(bf), wkT.astype(bf), wvT.astype(bf),
            wqT.astype(bf), wpT.astype(bf), expB.astype(bf), gb)


_state = None


def _get_state():
    global _state
    if _state is None:
        devs = jax.devices()[:NCORES]
        mesh = Mesh(np.asarray(devs), ("c",))
        shard = NamedSharding(mesh, P("c"))
        rep = NamedSharding(mesh, P())
        in_sh = (shard, shard) + (rep,) * 6
        from jax.experimental.shard_map import shard_map

        def _run(*args):
            return _bass_fn(*args)

        fn = jax.jit(shard_map(
            _run, mesh=mesh,
            in_specs=(P("c"), P("c")) + (P(),) * 6,
            out_specs=(P("c"),), check_rep=False))
        _state = (fn, in_sh)
    return _state


def _device_args(kw):
    _, in_sh = _get_state()
    host = _stage(kw)
    return tuple(jax.device_put(h, s) for h, s in zip(host, in_sh))


def kernel(**inputs):
    fn, _ = _get_state()
    (out,) = fn(*_device_args(inputs))
    out = np.asarray(out)                          # [8*1568, 768]
    return out.reshape(64, 196, 768)


def run_on_device(dargs):
    fn, _ = _get_state()
    return fn(*dargs)
